# revision 10
# baseline (speedup 1.0000x reference)
"""Trainium2 Bass kernel for channel self-attention (nn_CA_Module).

Reference (per batch item b, q = x[b] reshaped [C=64, N=65536]):
    att    = q @ q^T                                  [64, 64]
    att_sm = softmax(rowmax(att) - att, axis=-1)
           = exp(rowmin(att) - att) / rowsum(...)     (reversed softmax)
    out[b] = gamma * (att_sm @ q) + x[b]

Sharding: data-parallel over batch: 16 batch items -> 8 cores x 2.

Per-core layout: each batch item's q is stored in SBUF as [128, 32768]
with partition p = h*64 + c  (h = which half of N, c = channel).  The
Gram matrix is computed by PE-transposing 128-wide column chunks
(fp32, exact) and accumulating two 64-column matmuls per chunk into one
[64,64] PSUM accumulator (this folds the two halves' partial Grams).
The softmax weights (including gamma and the 1/rowsum normalization)
are folded into a block-diagonal [128,128] weight matrix W so that the
second matmul (W.T @ q16, fp16 full-rate) and the final fp32 residual
add produce the output directly in the same layout.
"""

import sys

if "/opt/trn_rl_repo" not in sys.path:
    sys.path.insert(0, "/opt/trn_rl_repo")

import numpy as np

B, C, H, W_ = 16, 64, 256, 256
N = H * W_            # 65536
HALF = N // 2         # 32768
N_CORES = 8
B_PER_CORE = B // N_CORES   # 2
GRAN = 2048           # granule width (fp32, 8KB/partition), 16 per batch item
NGRAN = HALF // GRAN  # 16
TCH = 128             # transpose chunk width
GROUP = 512           # psum-bank group: 4 transposes per group
MM2 = 512             # matmul2 free-dim chunk

_PROGRAM = None


def _build_program():
    """Build + compile the per-core Bacc program. Returns the nc object."""
    import concourse.bacc as bacc
    import concourse.tile as tile
    import concourse.mybir as mybir

    f32 = mybir.dt.float32
    f16 = mybir.dt.float16

    nc = bacc.Bacc("TRN2", target_bir_lowering=False, debug=False)
    X = nc.dram_tensor("x", [B_PER_CORE, C, N], f32, kind="ExternalInput").ap()
    G = nc.dram_tensor("gamma", [1], f32, kind="ExternalInput").ap()
    O = nc.dram_tensor("out", [B_PER_CORE, C, N], f32, kind="ExternalOutput").ap()

    with tile.TileContext(nc) as tc:
        with tc.tile_pool(name="xg", bufs=NGRAN + 2) as xg_pool, \
             tc.tile_pool(name="qt", bufs=3) as qt_pool, \
             tc.tile_pool(name="q16", bufs=3) as q16_pool, \
             tc.tile_pool(name="og", bufs=3) as og_pool, \
             tc.tile_pool(name="const", bufs=1) as const_pool, \
             tc.tile_pool(name="small", bufs=2) as small_pool, \
             tc.tile_pool(name="wsb", bufs=2) as w_pool, \
             tc.tile_pool(name="psqt", bufs=3, space="PSUM") as ps_qt, \
             tc.tile_pool(name="psatt", bufs=1, space="PSUM") as ps_att, \
             tc.tile_pool(name="psres", bufs=3, space="PSUM") as ps_res, \
             tc.tile_pool(name="psw", bufs=1, space="PSUM") as ps_w:

            # ---- prologue: constants ----
            g64 = const_pool.tile([C, 1], f32)
            nc.sync.dma_start(g64[:], G[None, :].to_broadcast((C, 1)))
            ones = const_pool.tile([128, 128], f32)
            nc.vector.memset(ones[:], 1.0)
            ident = const_pool.tile([128, 128], f32)
            # iota(p, f) = p - f ; keep where == 0 -> identity matrix
            nc.gpsimd.affine_select(
                ident[:], ones[:], pattern=[[-1, 128]],
                compare_op=mybir.AluOpType.is_equal, fill=0.0,
                base=0, channel_multiplier=1,
            )

            for b in range(B_PER_CORE):
                # [2, 64, 32768]: dims (h, c, m); partition p = h*64 + c
                xv = X[b].rearrange("c (h m) -> h c m", h=2)
                ov = O[b].rearrange("c (h m) -> h c m", h=2)

                # ---- phase 1: load + transpose + Gram ----
                att = ps_att.tile([C, C], f32)
                xg_tiles = []
                ngroups = NGRAN * (GRAN // GROUP)       # 64 groups of 512
                pend = []                               # software-pipeline lag
                gi = 0
                for g in range(NGRAN):
                    xg = xg_pool.tile([128, GRAN], f32)
                    nc.sync.dma_start(xg[:], xv[:, :, g * GRAN:(g + 1) * GRAN])
                    xg_tiles.append(xg)
                    for t in range(GRAN // GROUP):
                        qt_ps = ps_qt.tile([128, GROUP], f32)
                        for u in range(GROUP // TCH):
                            sl = xg[:, t * GROUP + u * TCH: t * GROUP + (u + 1) * TCH]
                            nc.tensor.transpose(
                                qt_ps[:, u * TCH:(u + 1) * TCH], sl, ident[:])
                        qt_sb = qt_pool.tile([128, GROUP], f32)
                        # balance PSUM->SBUF copies between DVE and ACT
                        if gi % 3 == 2:
                            nc.scalar.copy(qt_sb[:], qt_ps[:])
                        else:
                            nc.vector.tensor_copy(qt_sb[:], qt_ps[:])
                        pend.append(qt_sb)
                        # emit Gram matmuls with a 2-group lag so PE never
                        # stalls on the PSUM->SBUF copy
                        if len(pend) >= 3:
                            _emit_gram(nc, att, pend.pop(0), gi - 2, ngroups)
                        gi += 1
                while pend:
                    _emit_gram(nc, att, pend.pop(0), gi - len(pend), ngroups)

                # ---- softmax (reversed, stable via rowmin) + weight build ----
                mn = small_pool.tile([C, 1], f32)
                nc.vector.tensor_reduce(
                    out=mn[:], in_=att[:], axis=mybir.AxisListType.X,
                    op=mybir.AluOpType.min)
                e = small_pool.tile([C, C], f32)
                s = small_pool.tile([C, 1], f32)
                nc.scalar.activation(
                    e[:], att[:], mybir.ActivationFunctionType.Exp,
                    bias=mn[:], scale=-1.0, accum_out=s[:])
                rinv = small_pool.tile([C, 1], f32)
                nc.vector.reciprocal(rinv[:], s[:])
                gs = small_pool.tile([C, 1], f32)
                nc.vector.tensor_tensor(
                    out=gs[:], in0=rinv[:], in1=g64[:], op=mybir.AluOpType.mult)
                es = small_pool.tile([C, C], f32)
                nc.vector.tensor_scalar_mul(es[:], e[:], gs[:])

                # W blocks = es.T, placed block-diagonally. Regular matmuls
                # (es.T @ I) rather than transpose-mode: the transpose path
                # requires PSUM base partition 0, col-tiling does not.
                w_ps = ps_w.tile([128, 128], f32)
                nc.tensor.matmul(w_ps[0:C, 0:C], es[:], ident[0:C, 0:C],
                                 start=True, stop=True)
                nc.tensor.matmul(w_ps[C:128, C:128], es[:], ident[0:C, 0:C],
                                 start=True, stop=True, tile_position=(0, 64))
                # mm2 runs in fp16 (full PE rate; res error ~1e-4 relative):
                # weights cast to fp16 here, q cast per-granule below.
                w_sb = w_pool.tile([128, 128], f16)
                nc.vector.memset(w_sb[:], 0.0)
                nc.vector.tensor_copy(w_sb[0:C, 0:C], w_ps[0:C, 0:C])
                nc.vector.tensor_copy(w_sb[C:128, C:128], w_ps[C:128, C:128])

                # ---- phase 2: res = W.T @ q16 ; out = res + x ----
                for g in range(NGRAN):
                    og = og_pool.tile([128, GRAN], f32)
                    q16 = q16_pool.tile([128, GRAN], f16)
                    if g % 2 == 0:
                        nc.scalar.copy(q16[:], xg_tiles[g][:])
                    else:
                        nc.vector.tensor_copy(q16[:], xg_tiles[g][:])
                    for k in range(GRAN // MM2):
                        res = ps_res.tile([128, MM2], f32)
                        sl = xg_tiles[g][:, k * MM2:(k + 1) * MM2]
                        nc.tensor.matmul(
                            res[:], w_sb[:], q16[:, k * MM2:(k + 1) * MM2],
                            start=True, stop=True)
                        nc.vector.tensor_tensor(
                            out=og[:, k * MM2:(k + 1) * MM2], in0=res[:],
                            in1=sl, op=mybir.AluOpType.add)
                    nc.scalar.dma_start(ov[:, :, g * GRAN:(g + 1) * GRAN], og[:])

    nc.compile()
    return nc


def _emit_gram(nc, att, qt_sb, gi, ngroups):
    """Accumulate Gram contributions of one 512-wide transposed group.
    qt_sb columns: 0:64 = half-0 channels, 64:128 = half-1 channels
    (per 128-chunk). Two 64-col matmuls fold both halves into att."""
    nchunks = GROUP // TCH
    for u in range(nchunks):
        qh = qt_sb[:, u * TCH:(u + 1) * TCH]
        first = gi == 0 and u == 0
        last = gi == ngroups - 1 and u == nchunks - 1
        nc.tensor.matmul(att[:], qh[:, 0:C], qh[:, 0:C],
                         start=first, stop=False)
        nc.tensor.matmul(att[:], qh[:, C:128], qh[:, C:128],
                         start=False, stop=last)


def _get_program():
    global _PROGRAM
    if _PROGRAM is None:
        _PROGRAM = _build_program()
    return _PROGRAM


def kernel(x: np.ndarray, gamma: np.ndarray) -> np.ndarray:
    from concourse.bass_utils import run_bass_kernel_spmd

    nc = _get_program()
    x = np.ascontiguousarray(x, dtype=np.float32)
    gamma = np.ascontiguousarray(gamma, dtype=np.float32)
    xr = x.reshape(B, C, N)
    in_maps = [
        {"x": xr[i * B_PER_CORE:(i + 1) * B_PER_CORE], "gamma": gamma}
        for i in range(N_CORES)
    ]
    res = run_bass_kernel_spmd(nc, in_maps, list(range(N_CORES)))
    out = np.concatenate([res.results[i]["out"] for i in range(N_CORES)], axis=0)
    return out.reshape(B, C, H, W_)


# revision 11
# speedup vs baseline: 3.9028x; 3.9028x over previous
"""Trainium2 Bass kernel for channel self-attention (nn_CA_Module).

Reference (per batch item b, q = x[b] reshaped [C=64, N=65536]):
    att    = q @ q^T                                  [64, 64]
    att_sm = softmax(rowmax(att) - att, axis=-1)
           = exp(rowmin(att) - att) / rowsum(...)     (reversed softmax)
    out[b] = gamma * (att_sm @ q) + x[b]

Sharding: data-parallel over batch: 16 batch items -> 8 cores x 2.

Per-core layout: each batch item's q is stored in SBUF as [128, 32768]
with partition p = h*64 + c  (h = which half of N, c = channel).  The
Gram matrix is computed by PE-transposing 128-wide column chunks
(fp32, exact) and accumulating two 64-column matmuls per chunk into one
[64,64] PSUM accumulator (this folds the two halves' partial Grams).
The softmax weights (including gamma and the 1/rowsum normalization)
are folded into a block-diagonal [128,128] weight matrix W so that the
second matmul (W.T @ q16, fp16 full-rate) and the final fp32 residual
add produce the output directly in the same layout.

Emission interleaves batch b1's phase 1 between batch b0's phase-2
granules so PE/DMA/DVE stay busy across the phase boundary.
"""

import sys

if "/opt/trn_rl_repo" not in sys.path:
    sys.path.insert(0, "/opt/trn_rl_repo")

import numpy as np

B, C, H, W_ = 16, 64, 256, 256
N = H * W_            # 65536
HALF = N // 2         # 32768
N_CORES = 8
B_PER_CORE = B // N_CORES   # 2
GRAN = 2048           # granule width (fp32, 8KB/partition), 16 per batch item
NGRAN = HALF // GRAN  # 16
TCH = 128             # transpose chunk width
GROUP = 512           # psum-bank group: 4 transposes per group
MM2 = 512             # matmul2 free-dim chunk
GRAM_LAG = 2          # groups of lag between PSUM->SBUF copy and Gram use

_PROGRAM = None


class _Ctx:
    pass


def _build_program(reps=1):
    """Build + compile the per-core Bacc program. Returns the nc object.

    reps > 1 wraps the whole body in a hardware loop that recomputes the
    same outputs; used only for wall-clock timing (amortizes dispatch RTT).
    """
    import contextlib
    import concourse.bacc as bacc
    import concourse.tile as tile
    import concourse.mybir as mybir

    f32 = mybir.dt.float32
    f16 = mybir.dt.float16

    nc = bacc.Bacc("TRN2", target_bir_lowering=False, debug=False)
    X = nc.dram_tensor("x", [B_PER_CORE, C, N], f32, kind="ExternalInput").ap()
    G = nc.dram_tensor("gamma", [1], f32, kind="ExternalInput").ap()
    O = nc.dram_tensor("out", [B_PER_CORE, C, N], f32, kind="ExternalOutput").ap()

    c = _Ctx()
    c.mybir = mybir
    c.f32, c.f16 = f32, f16

    with tile.TileContext(nc) as tc:
        with tc.tile_pool(name="xg", bufs=NGRAN + 2) as c.xg_pool, \
             tc.tile_pool(name="qt", bufs=GRAM_LAG + 1) as c.qt_pool, \
             tc.tile_pool(name="q16", bufs=3) as c.q16_pool, \
             tc.tile_pool(name="og", bufs=3) as c.og_pool, \
             tc.tile_pool(name="const", bufs=1) as const_pool, \
             tc.tile_pool(name="small", bufs=2) as c.small_pool, \
             tc.tile_pool(name="wsb", bufs=2) as c.w_pool, \
             tc.tile_pool(name="psqt", bufs=GRAM_LAG + 1, space="PSUM") as c.ps_qt, \
             tc.tile_pool(name="psatt", bufs=1, space="PSUM") as c.ps_att, \
             tc.tile_pool(name="psres", bufs=3, space="PSUM") as c.ps_res, \
             tc.tile_pool(name="psw", bufs=1, space="PSUM") as c.ps_w:

            # ---- prologue: constants ----
            c.g64 = const_pool.tile([C, 1], f32)
            nc.sync.dma_start(c.g64[:], G[None, :].to_broadcast((C, 1)))
            ones = const_pool.tile([128, 128], f32)
            nc.vector.memset(ones[:], 1.0)
            c.ident = const_pool.tile([128, 128], f32)
            # iota(p, f) = p - f ; keep where == 0 -> identity matrix
            nc.gpsimd.affine_select(
                c.ident[:], ones[:], pattern=[[-1, 128]],
                compare_op=mybir.AluOpType.is_equal, fill=0.0,
                base=0, channel_multiplier=1,
            )

            views = []
            for b in range(B_PER_CORE):
                # [2, 64, 32768]: dims (h, c, m); partition p = h*64 + c
                views.append((X[b].rearrange("c (h m) -> h c m", h=2),
                              O[b].rearrange("c (h m) -> h c m", h=2)))

            loop_cm = tc.For_i(0, reps, 1) if reps > 1 else contextlib.nullcontext()
            with loop_cm:
                states = [_Ctx() for _ in range(B_PER_CORE)]
                # batch 0 phase 1 (alone: nothing to overlap yet)
                _start_phase1(c, nc, states[0])
                for g in range(NGRAN):
                    _emit_phase1_granule(c, nc, states[0], views[0][0], g)
                _finish_phase1(c, nc, states[0])
                _emit_softmax(c, nc, states[0])
                # batch 0 phase 2 interleaved with batch 1 phase 1
                _start_phase1(c, nc, states[1])
                for g in range(NGRAN):
                    _emit_phase2_granule(c, nc, states[0], views[0][1], g)
                    _emit_phase1_granule(c, nc, states[1], views[1][0], g)
                _finish_phase1(c, nc, states[1])
                _emit_softmax(c, nc, states[1])
                for g in range(NGRAN):
                    _emit_phase2_granule(c, nc, states[1], views[1][1], g)

    nc.compile()
    return nc


def _start_phase1(c, nc, st):
    st.att = c.ps_att.tile([C, C], c.f32)
    st.xg_tiles = []
    st.pend = []
    st.gi = 0
    st.ngroups = NGRAN * (GRAN // GROUP)


def _emit_phase1_granule(c, nc, st, xv, g):
    """Load granule g, transpose its 16 chunks, accumulate the Gram."""
    xg = c.xg_pool.tile([128, GRAN], c.f32)
    nc.sync.dma_start(xg[:], xv[:, :, g * GRAN:(g + 1) * GRAN])
    st.xg_tiles.append(xg)
    for t in range(GRAN // GROUP):
        qt_ps = c.ps_qt.tile([128, GROUP], c.f32)
        for u in range(GROUP // TCH):
            sl = xg[:, t * GROUP + u * TCH: t * GROUP + (u + 1) * TCH]
            nc.tensor.transpose(qt_ps[:, u * TCH:(u + 1) * TCH], sl, c.ident[:])
        qt_sb = c.qt_pool.tile([128, GROUP], c.f32)
        # balance PSUM->SBUF copies between DVE and ACT
        if st.gi % 3 == 2:
            nc.scalar.copy(qt_sb[:], qt_ps[:])
        else:
            nc.vector.tensor_copy(qt_sb[:], qt_ps[:])
        st.pend.append(qt_sb)
        # Gram matmuls lag the copies so PE never waits on DVE/ACT
        if len(st.pend) > GRAM_LAG:
            _emit_gram(c, nc, st.att, st.pend.pop(0), st.gi - GRAM_LAG, st.ngroups)
        st.gi += 1


def _finish_phase1(c, nc, st):
    while st.pend:
        _emit_gram(c, nc, st.att, st.pend.pop(0), st.gi - len(st.pend), st.ngroups)


def _emit_gram(c, nc, att, qt_sb, gi, ngroups):
    """Accumulate Gram contributions of one 512-wide transposed group.
    Per 128-chunk, columns 0:64 are half-0 channels at its n-range and
    columns 64:128 half-1 channels; two 64-col matmuls fold both halves."""
    nchunks = GROUP // TCH
    for u in range(nchunks):
        qh = qt_sb[:, u * TCH:(u + 1) * TCH]
        first = gi == 0 and u == 0
        last = gi == ngroups - 1 and u == nchunks - 1
        nc.tensor.matmul(att[:], qh[:, 0:C], qh[:, 0:C],
                         start=first, stop=False)
        nc.tensor.matmul(att[:], qh[:, C:128], qh[:, C:128],
                         start=False, stop=last)


def _emit_softmax(c, nc, st):
    """Reversed softmax (stable via rowmin) + gamma/normalization folded
    into a block-diagonal fp16 weight matrix W = diag(es.T, es.T)."""
    mybir, f32, f16 = c.mybir, c.f32, c.f16
    mn = c.small_pool.tile([C, 1], f32)
    nc.vector.tensor_reduce(out=mn[:], in_=st.att[:],
                            axis=mybir.AxisListType.X, op=mybir.AluOpType.min)
    e = c.small_pool.tile([C, C], f32)
    s = c.small_pool.tile([C, 1], f32)
    nc.scalar.activation(e[:], st.att[:], mybir.ActivationFunctionType.Exp,
                         bias=mn[:], scale=-1.0, accum_out=s[:])
    rinv = c.small_pool.tile([C, 1], f32)
    nc.vector.reciprocal(rinv[:], s[:])
    gs = c.small_pool.tile([C, 1], f32)
    nc.vector.tensor_tensor(out=gs[:], in0=rinv[:], in1=c.g64[:],
                            op=mybir.AluOpType.mult)
    es = c.small_pool.tile([C, C], f32)
    nc.vector.tensor_scalar_mul(es[:], e[:], gs[:])

    # W blocks = es.T via regular matmuls (es.T @ I): the transpose-mode
    # path requires PSUM base partition 0, col-tiling does not.
    w_ps = c.ps_w.tile([128, 128], f32)
    nc.tensor.matmul(w_ps[0:C, 0:C], es[:], c.ident[0:C, 0:C],
                     start=True, stop=True)
    nc.tensor.matmul(w_ps[C:128, C:128], es[:], c.ident[0:C, 0:C],
                     start=True, stop=True, tile_position=(0, 64))
    st.w_sb = c.w_pool.tile([128, 128], f16)
    nc.vector.memset(st.w_sb[:], 0.0)
    nc.vector.tensor_copy(st.w_sb[0:C, 0:C], w_ps[0:C, 0:C])
    nc.vector.tensor_copy(st.w_sb[C:128, C:128], w_ps[C:128, C:128])


def _emit_phase2_granule(c, nc, st, ov, g):
    """res = W.T @ q16 (fp16 matmul); out = res + x; store granule."""
    mybir, f32, f16 = c.mybir, c.f32, c.f16
    og = c.og_pool.tile([128, GRAN], f32)
    q16 = c.q16_pool.tile([128, GRAN], f16)
    if g % 2 == 0:
        nc.scalar.copy(q16[:], st.xg_tiles[g][:])
    else:
        nc.vector.tensor_copy(q16[:], st.xg_tiles[g][:])
    for k in range(GRAN // MM2):
        res = c.ps_res.tile([128, MM2], f32)
        sl = st.xg_tiles[g][:, k * MM2:(k + 1) * MM2]
        nc.tensor.matmul(res[:], st.w_sb[:], q16[:, k * MM2:(k + 1) * MM2],
                         start=True, stop=True)
        nc.vector.tensor_tensor(out=og[:, k * MM2:(k + 1) * MM2], in0=res[:],
                                in1=sl, op=mybir.AluOpType.add)
    nc.scalar.dma_start(ov[:, :, g * GRAN:(g + 1) * GRAN], og[:])


def _get_program():
    global _PROGRAM
    if _PROGRAM is None:
        _PROGRAM = _build_program()
    return _PROGRAM


def kernel(x: np.ndarray, gamma: np.ndarray) -> np.ndarray:
    from concourse.bass_utils import run_bass_kernel_spmd

    nc = _get_program()
    x = np.ascontiguousarray(x, dtype=np.float32)
    gamma = np.ascontiguousarray(gamma, dtype=np.float32)
    xr = x.reshape(B, C, N)
    in_maps = [
        {"x": xr[i * B_PER_CORE:(i + 1) * B_PER_CORE], "gamma": gamma}
        for i in range(N_CORES)
    ]
    res = run_bass_kernel_spmd(nc, in_maps, list(range(N_CORES)))
    out = np.concatenate([res.results[i]["out"] for i in range(N_CORES)], axis=0)
    return out.reshape(B, C, H, W_)


# revision 12
# speedup vs baseline: 11.0609x; 2.8341x over previous
"""Trainium2 Bass kernel for channel self-attention (nn_CA_Module).

Reference (per batch item b, q = x[b] reshaped [C=64, N=65536]):
    att    = q @ q^T                                  [64, 64]
    att_sm = softmax(rowmax(att) - att, axis=-1)
           = exp(rowmin(att) - att) / rowsum(...)     (reversed softmax)
    out[b] = gamma * (att_sm @ q) + x[b]

Sharding: data-parallel over batch: 16 batch items -> 8 cores x 2.

Per-core layout: each batch item's q is stored in SBUF as [128, 32768]
with partition p = h*64 + c  (h = which half of N, c = channel).  The
Gram matrix is computed by PE-transposing 128-wide column chunks
(fp32, exact) and accumulating two 64-column matmuls per chunk into one
[64,64] PSUM accumulator (this folds the two halves' partial Grams).
The softmax weights (including gamma and the 1/rowsum normalization)
are folded into a block-diagonal [128,128] weight matrix W so that the
second matmul (W.T @ q16, fp16 full-rate) and the final fp32 residual
add produce the output directly in the same layout.

Emission interleaves batch b1's phase 1 between batch b0's phase-2
granules so PE/DMA/DVE stay busy across the phase boundary.
"""

import sys

if "/opt/trn_rl_repo" not in sys.path:
    sys.path.insert(0, "/opt/trn_rl_repo")

import numpy as np

B, C, H, W_ = 16, 64, 256, 256
N = H * W_            # 65536
HALF = N // 2         # 32768
N_CORES = 8
B_PER_CORE = B // N_CORES   # 2
GRAN = 2048           # granule width (fp32, 8KB/partition), 16 per batch item
NGRAN = HALF // GRAN  # 16
TCH = 128             # transpose chunk width
GROUP = 512           # psum-bank group: 4 transposes per group
MM2 = 512             # matmul2 free-dim chunk
GRAM_LAG = 2          # groups of lag between PSUM->SBUF copy and Gram use

_PROGRAM = None


class _Ctx:
    pass


def _build_program(reps=1):
    """Build + compile the per-core Bacc program. Returns the nc object.

    reps > 1 wraps the whole body in a hardware loop that recomputes the
    same outputs; used only for wall-clock timing (amortizes dispatch RTT).
    """
    import contextlib
    import concourse.bacc as bacc
    import concourse.tile as tile
    import concourse.mybir as mybir

    f32 = mybir.dt.float32
    f16 = mybir.dt.float16

    nc = bacc.Bacc("TRN2", target_bir_lowering=False, debug=False)
    X = nc.dram_tensor("x", [B_PER_CORE, C, N], f32, kind="ExternalInput").ap()
    G = nc.dram_tensor("gamma", [1], f32, kind="ExternalInput").ap()
    O = nc.dram_tensor("out", [B_PER_CORE, C, N], f32, kind="ExternalOutput").ap()

    c = _Ctx()
    c.mybir = mybir
    c.f32, c.f16 = f32, f16
    c.ring_i = 0

    with tile.TileContext(nc) as tc:
        with tc.tile_pool(name="xg", bufs=NGRAN + 2) as c.xg_pool, \
             tc.tile_pool(name="qt", bufs=GRAM_LAG + 1) as c.qt_pool, \
             tc.tile_pool(name="q16", bufs=3) as c.q16_pool, \
             tc.tile_pool(name="og", bufs=3) as c.og_pool, \
             tc.tile_pool(name="const", bufs=1) as const_pool, \
             tc.tile_pool(name="small", bufs=2) as c.small_pool, \
             tc.tile_pool(name="wsb", bufs=2) as c.w_pool, \
             tc.tile_pool(name="psqt", bufs=GRAM_LAG + 1, space="PSUM") as c.ps_qt, \
             tc.tile_pool(name="psatt", bufs=1, space="PSUM") as c.ps_att, \
             tc.tile_pool(name="psres", bufs=3, space="PSUM") as c.ps_res, \
             tc.tile_pool(name="psw", bufs=1, space="PSUM") as c.ps_w:

            # ---- prologue: constants ----
            c.g64 = const_pool.tile([C, 1], f32)
            nc.sync.dma_start(c.g64[:], G[None, :].to_broadcast((C, 1)))
            ones = const_pool.tile([128, 128], f32)
            nc.vector.memset(ones[:], 1.0)
            c.ident = const_pool.tile([128, 128], f32)
            # iota(p, f) = p - f ; keep where == 0 -> identity matrix
            nc.gpsimd.affine_select(
                c.ident[:], ones[:], pattern=[[-1, 128]],
                compare_op=mybir.AluOpType.is_equal, fill=0.0,
                base=0, channel_multiplier=1,
            )

            views = []
            for b in range(B_PER_CORE):
                # [2, 64, 32768]: dims (h, c, m); partition p = h*64 + c
                views.append((X[b].rearrange("c (h m) -> h c m", h=2),
                              O[b].rearrange("c (h m) -> h c m", h=2)))

            loop_cm = tc.For_i(0, reps, 1) if reps > 1 else contextlib.nullcontext()
            with loop_cm:
                states = [_Ctx() for _ in range(B_PER_CORE)]
                # batch 0 phase 1 (alone: nothing to overlap yet)
                _start_phase1(c, nc, states[0])
                for g in range(NGRAN):
                    _emit_phase1_granule(c, nc, states[0], views[0][0], g)
                _finish_phase1(c, nc, states[0])
                _emit_softmax(c, nc, states[0])
                # batch 0 phase 2 interleaved with batch 1 phase 1
                _start_phase1(c, nc, states[1])
                for g in range(NGRAN):
                    _emit_phase2_granule(c, nc, states[0], views[0][1], g)
                    _emit_phase1_granule(c, nc, states[1], views[1][0], g)
                _finish_phase1(c, nc, states[1])
                _emit_softmax(c, nc, states[1])
                for g in range(NGRAN):
                    _emit_phase2_granule(c, nc, states[1], views[1][1], g)

    nc.compile()
    return nc


def _ring(c, nc):
    engines = (nc.sync, nc.scalar, nc.gpsimd)
    e = engines[c.ring_i % 3]
    c.ring_i += 1
    return e


def _start_phase1(c, nc, st):
    st.att = c.ps_att.tile([C, C], c.f32)
    st.xg_tiles = []
    st.pend = []
    st.gi = 0
    st.ngroups = NGRAN * (GRAN // GROUP)


def _emit_phase1_granule(c, nc, st, xv, g):
    """Load granule g, transpose its 16 chunks, accumulate the Gram.

    Loads are two 2D-AP DMAs (one per half): 3D APs run ~4x slower on
    this runtime (~1.1us per outer row), and round-robin over the three
    DMA-capable engines {sync, scalar, gpsimd} balances ring bandwidth.
    """
    xg = c.xg_pool.tile([128, GRAN], c.f32)
    sl = slice(g * GRAN, (g + 1) * GRAN)
    for h in range(2):
        _ring(c, nc).dma_start(xg[h * 64:(h + 1) * 64, :], xv[h][:, sl])
    st.xg_tiles.append(xg)
    for t in range(GRAN // GROUP):
        qt_ps = c.ps_qt.tile([128, GROUP], c.f32)
        for u in range(GROUP // TCH):
            sl = xg[:, t * GROUP + u * TCH: t * GROUP + (u + 1) * TCH]
            nc.tensor.transpose(qt_ps[:, u * TCH:(u + 1) * TCH], sl, c.ident[:])
        qt_sb = c.qt_pool.tile([128, GROUP], c.f32)
        # balance PSUM->SBUF copies between DVE and ACT
        if st.gi % 3 == 2:
            nc.scalar.copy(qt_sb[:], qt_ps[:])
        else:
            nc.vector.tensor_copy(qt_sb[:], qt_ps[:])
        st.pend.append(qt_sb)
        # Gram matmuls lag the copies so PE never waits on DVE/ACT
        if len(st.pend) > GRAM_LAG:
            _emit_gram(c, nc, st.att, st.pend.pop(0), st.gi - GRAM_LAG, st.ngroups)
        st.gi += 1


def _finish_phase1(c, nc, st):
    while st.pend:
        _emit_gram(c, nc, st.att, st.pend.pop(0), st.gi - len(st.pend), st.ngroups)


def _emit_gram(c, nc, att, qt_sb, gi, ngroups):
    """Accumulate Gram contributions of one 512-wide transposed group.
    Per 128-chunk, columns 0:64 are half-0 channels at its n-range and
    columns 64:128 half-1 channels; two 64-col matmuls fold both halves."""
    nchunks = GROUP // TCH
    for u in range(nchunks):
        qh = qt_sb[:, u * TCH:(u + 1) * TCH]
        first = gi == 0 and u == 0
        last = gi == ngroups - 1 and u == nchunks - 1
        nc.tensor.matmul(att[:], qh[:, 0:C], qh[:, 0:C],
                         start=first, stop=False)
        nc.tensor.matmul(att[:], qh[:, C:128], qh[:, C:128],
                         start=False, stop=last)


def _emit_softmax(c, nc, st):
    """Reversed softmax (stable via rowmin) + gamma/normalization folded
    into a block-diagonal fp16 weight matrix W = diag(es.T, es.T)."""
    mybir, f32, f16 = c.mybir, c.f32, c.f16
    mn = c.small_pool.tile([C, 1], f32)
    nc.vector.tensor_reduce(out=mn[:], in_=st.att[:],
                            axis=mybir.AxisListType.X, op=mybir.AluOpType.min)
    e = c.small_pool.tile([C, C], f32)
    s = c.small_pool.tile([C, 1], f32)
    nc.scalar.activation(e[:], st.att[:], mybir.ActivationFunctionType.Exp,
                         bias=mn[:], scale=-1.0, accum_out=s[:])
    rinv = c.small_pool.tile([C, 1], f32)
    nc.vector.reciprocal(rinv[:], s[:])
    gs = c.small_pool.tile([C, 1], f32)
    nc.vector.tensor_tensor(out=gs[:], in0=rinv[:], in1=c.g64[:],
                            op=mybir.AluOpType.mult)
    es = c.small_pool.tile([C, C], f32)
    nc.vector.tensor_scalar_mul(es[:], e[:], gs[:])

    # W blocks = es.T via regular matmuls (es.T @ I): the transpose-mode
    # path requires PSUM base partition 0, col-tiling does not.
    w_ps = c.ps_w.tile([128, 128], f32)
    nc.tensor.matmul(w_ps[0:C, 0:C], es[:], c.ident[0:C, 0:C],
                     start=True, stop=True)
    nc.tensor.matmul(w_ps[C:128, C:128], es[:], c.ident[0:C, 0:C],
                     start=True, stop=True, tile_position=(0, 64))
    st.w_sb = c.w_pool.tile([128, 128], f16)
    nc.vector.memset(st.w_sb[:], 0.0)
    nc.vector.tensor_copy(st.w_sb[0:C, 0:C], w_ps[0:C, 0:C])
    nc.vector.tensor_copy(st.w_sb[C:128, C:128], w_ps[C:128, C:128])


def _emit_phase2_granule(c, nc, st, ov, g):
    """res = W.T @ q16 (fp16 matmul); out = res + x; store granule."""
    mybir, f32, f16 = c.mybir, c.f32, c.f16
    og = c.og_pool.tile([128, GRAN], f32)
    q16 = c.q16_pool.tile([128, GRAN], f16)
    if g % 2 == 0:
        nc.scalar.copy(q16[:], st.xg_tiles[g][:])
    else:
        nc.vector.tensor_copy(q16[:], st.xg_tiles[g][:])
    for k in range(GRAN // MM2):
        res = c.ps_res.tile([128, MM2], f32)
        sl = st.xg_tiles[g][:, k * MM2:(k + 1) * MM2]
        nc.tensor.matmul(res[:], st.w_sb[:], q16[:, k * MM2:(k + 1) * MM2],
                         start=True, stop=True)
        nc.vector.tensor_tensor(out=og[:, k * MM2:(k + 1) * MM2], in0=res[:],
                                in1=sl, op=mybir.AluOpType.add)
    sl = slice(g * GRAN, (g + 1) * GRAN)
    for h in range(2):
        _ring(c, nc).dma_start(ov[h][:, sl], og[h * 64:(h + 1) * 64, :])


def _get_program():
    global _PROGRAM
    if _PROGRAM is None:
        _PROGRAM = _build_program()
    return _PROGRAM


def kernel(x: np.ndarray, gamma: np.ndarray) -> np.ndarray:
    from concourse.bass_utils import run_bass_kernel_spmd

    nc = _get_program()
    x = np.ascontiguousarray(x, dtype=np.float32)
    gamma = np.ascontiguousarray(gamma, dtype=np.float32)
    xr = x.reshape(B, C, N)
    in_maps = [
        {"x": xr[i * B_PER_CORE:(i + 1) * B_PER_CORE], "gamma": gamma}
        for i in range(N_CORES)
    ]
    res = run_bass_kernel_spmd(nc, in_maps, list(range(N_CORES)))
    out = np.concatenate([res.results[i]["out"] for i in range(N_CORES)], axis=0)
    return out.reshape(B, C, H, W_)
